# revision 59
# baseline (speedup 1.0000x reference)
"""Trainium2 Bass kernel for GQA attention (B=2, S=2048, D=1024, H=16, HKV=4).

Sharding: 8 cores = batch(2) x kv-group(4). Each core handles one batch and
one KV head group (4 query heads + 1 KV head), computes attention plus its
partial slice of the output projection (row-parallel wo); the host sums the
4 tensor-parallel partials per batch (partials are bf16, summed in f32).
No device collectives.

Per-core device kernel (matmul operands bf16, fp32 PSUM accumulation).
Work units:
  - proj(g, sbl): QKV projection for s-block 4g+sbl (8 dc-chunk matmuls into
    one PSUM accumulator, then PSUM->SBUF copy of q/k to qkv and v to vb).
  - rope(g): RoPE for group g's 4 s-blocks on DVE (weights pre-permuted so
    each head is [32 real | 32 imag]).
  - trq(g, half)/trk(g): PE-transpose q -> qT and k -> kT2 (k duplicated to
    partitions 64-127 for two-head row tiling).
  - chunk(pair, kb, sp): scores S^T = kT.T @ qT for one k-block over q in
    [512*sp, 512*(sp+1)), both heads of a pair in one [128, 2w] PSUM tile;
    diagonal blocks masked by accumulating identity @ (-1e9 upper-triangle);
    one exp per chunk on ACT (scale=1/8 folded; no max-subtraction).
  - pv(pair, qb): po[q=128, 2*65] accumulates eS[k,q].T @ v[k, 65] over
    kb <= qb with a ones column collecting the softmax denominator, then
    DVE reciprocal + tensor_scalar normalize, PE-transpose to attnT.
  - stage3(sb): y = attnT.T @ woT for s-block sb, PSUM->SBUF copy, DMA out.

The emission order is produced by a greedy static scheduler that models
per-engine clocks (PE/ACT/DVE/DMA) and interleaves score chunks with all
other PE work so the ACT exp stream (the ~60us serial floor) runs
continuously from ~8us on, while PE (~84us of matmul work) never waits on
exp except through the 2-deep score PSUM pool.
"""

import numpy as np
import ml_dtypes

B, S, D = 2, 2048, 1024
H, HKV, HD = 16, 4, 64
REP = H // HKV  # 4 query heads per kv head
N_CORES = 8
NSB = S // 128  # 16 s-blocks
NDC = D // 128  # 8 d-chunks
QKV = REP * HD + 2 * HD  # 384 projected dims per core
NSP = 4  # q subpasses of 512
BF16 = ml_dtypes.bfloat16

_CACHE = {}


def _w_of(kb, sp):
    """q-width of the (kb, sp) score chunk (per head)."""
    return (sp + 1) * 512 - max(sp * 512, 128 * kb)


def _spoff():
    """eSb slot offsets: off[sp][(pair, kb)] in the single resident buffer."""
    offs = []
    cum = 0
    for sp in range(NSP):
        off = {}
        for pair in range(2):
            for kb in range(4 * sp + 4):
                off[(pair, kb)] = cum
                cum += 2 * _w_of(kb, sp)
        offs.append(off)
    return offs, cum


SPOFF, EBUF_SIZE = _spoff()

# ---- static greedy schedule ------------------------------------------------
# Modeled per-instruction costs (ns), matching the TRN2 cost model closely
# enough to order emissions well.
PEC = 0.4166


def _chunk_pe(kb, sp):
    w = _w_of(kb, sp)
    diag = 128 * kb == max(sp * 512, 128 * kb)
    return 2 * w * PEC + (53 if diag else 0)


def _chunk_act(kb, sp):
    return 2 * _w_of(kb, sp) * 0.833 + 190


def _schedule():
    """Return the emission order as a list of ('kind', args) tuples."""
    order = []
    # modeled clocks
    t = {'PE': 0.0, 'ACT': 2500.0, 'DVE': 0.0}
    # DMA availability (modeled from the serialized prelude transfer queue):
    # emitting a proj before its x strip lands would stall every later PE op
    # behind it in program order, so gate hard on these
    x_ready = {0: 4500.0, 1: 10000.0, 2: 13000.0, 3: 16000.0}

    proj_done = {}     # (g, sbl) -> emitted
    rope_done = set()
    trq_done = set()   # g
    trk_done = set()   # g
    chunks_emit = {}   # (pair, kb, sp) -> modeled exp end time
    pv_done = set()    # (pair, qb): FIN emitted (attnT written)
    st3_done = set()
    pending_fin = []   # pv fins waiting to be emitted one PE-item later

    def flush_fin(force=False):
        while pending_fin and (force or len(pending_fin) >= 2):
            order.append(('pvf', pending_fin.pop(0)))

    def note_pe_item():
        """Called after each PE-occupying emission: release one lagging fin
        so its transpose lands behind fresh PE work instead of a DVE wait."""
        if pending_fin:
            order.append(('pvf', pending_fin.pop(0)))

    # chunk stream order: sp ascending, kb ascending, pairs interleaved;
    # late-sp2 chunks are interleaved into early sp3 so their PVs (8-11)
    # become available as filler inside the long sp3 exp stretch
    chunk_list = []
    for sp in range(2):
        for kb in range(4 * sp + 4):
            for pair in range(2):
                chunk_list.append((pair, kb, sp))
    for kb in range(8):
        for pair in range(2):
            chunk_list.append((pair, kb, 2))
    late2 = [(pair, kb, 2) for kb in range(8, 12) for pair in range(2)]
    sp3 = [(pair, kb, 3) for kb in range(16) for pair in range(2)]
    while late2 or sp3:
        if late2:
            chunk_list.append(late2.pop(0))
        for _ in range(3):
            if sp3:
                chunk_list.append(sp3.pop(0))
    ci = 0

    def stage1_ready(g):
        return all((g, s) in proj_done for s in range(4))

    def emit_stage1_piece():
        """Emit the next stage1 piece (proj/rope/tr) if one is ready.
        Returns PE-time consumed (0 for DVE-only pieces)."""
        nonlocal order
        for g in range(4):
            for sbl in range(4):
                if (g, sbl) not in proj_done:
                    if t['PE'] < x_ready[g]:
                        return None  # inputs not there yet; don't stall PE
                    proj_done[(g, sbl)] = True
                    order.append(('proj', (g, sbl)))
                    note_pe_item()
                    t['PE'] += 8 * 384 * PEC
                    return 8 * 384 * PEC
            if g not in rope_done:
                rope_done.add(g)
                order.append(('rope', (g,)))
                t['DVE'] += 12 * 320
                return 0.0
            if g not in trq_done:
                trq_done.add(g)
                trk_done.add(g)
                order.append(('trq', (g, 0)))
                order.append(('trq', (g, 1)))
                order.append(('trk', (g,)))
                t['PE'] += 12 * 53
                return 12 * 53
        return None

    def pv_ready(pair, qb):
        sp = qb // 4
        return ((pair, qb) not in pv_done
                and all((pair, kb, sp) in chunks_emit
                        for kb in range(qb + 1)))

    def emit_pv(pair, qb):
        order.append(('pvm', (pair, qb, ci >= len(chunk_list))))
        note_pe_item()
        pending_fin.append((pair, qb))
        pv_done.add((pair, qb))
        dt = 2 * (qb + 1) * 65 * PEC + 53
        t['PE'] += dt
        t['DVE'] += 700
        return dt

    def st3_ready(sb):
        return (sb not in st3_done
                and (0, sb) in pv_done and (1, sb) in pv_done
                and (0, sb) not in pending_fin
                and (1, sb) not in pending_fin)

    def emit_st3(sb, split):
        order.append(('st3', (sb, split)))
        note_pe_item()
        st3_done.add(sb)
        t['PE'] += 4 * 512 * PEC
        t['DVE'] += 660
        return 4 * 512 * PEC

    # ACT-surplus of each chunk: filler PE-time the schedule must supply
    # alongside it for PE not to block on the 2-deep psS pool
    surplus = [max(0.0, _chunk_act(kb, sp) - _chunk_pe(kb, sp))
               for (_, kb, sp) in chunk_list]
    suffix_surplus = [0.0] * (len(chunk_list) + 1)
    for i in range(len(chunk_list) - 1, -1, -1):
        suffix_surplus[i] = suffix_surplus[i + 1] + surplus[i]

    def flex_left():
        """PE-time of unemitted PV + stage3 work."""
        tot = 0.0
        for qb in range(16):
            for pair in range(2):
                if (pair, qb) not in pv_done:
                    tot += 2 * (qb + 1) * 65 * PEC + 53
        tot += (16 - len(st3_done)) * 4 * 512 * PEC
        return tot

    def emit_filler():
        """Pick one filler item; returns PE ns consumed or None.
        PV/stage3 are hoarded unless we hold more than future chunks need."""
        r = emit_stage1_piece()
        if r is not None:
            return r
        if flex_left() <= suffix_surplus[ci]:
            return None  # reserve PV/stage3 for later ACT-bound stretches
        # PV with the lowest qb ready
        for qb in range(16):
            for pair in range(2):
                if (pair, qb) not in pv_done and pv_ready(pair, qb):
                    return emit_pv(pair, qb)
        # stage3 for blocks whose both PVs are done (keep one-block lag)
        ready3 = [sb for sb in range(16) if st3_ready(sb)]
        if len(ready3) >= 2 or (ready3 and ci >= len(chunk_list)):
            return emit_st3(ready3[0], False)
        return None

    # kick off: g0 proj with dc0-3/dc4-7 split chains over 3 accumulators
    # (sbl 0-2) so PE starts as soon as the first x/wcat halves land; sbl3
    # runs whole once everything is resident
    for sbl in range(3):
        order.append(('proja', (0, sbl)))
    for sbl in range(3):
        order.append(('projb', (0, sbl)))
    order.append(('proj', (0, 3)))
    for sbl in range(4):
        proj_done[(0, sbl)] = True
    t['PE'] = x_ready[0] + 4 * 8 * 384 * PEC
    while not (stage1_ready(0) and 0 in trq_done):
        r = emit_stage1_piece()
        if r is None:
            t['PE'] += 100.0  # modeled wait for DMA

    while ci < len(chunk_list):
        pair, kb, sp = chunk_list[ci]
        # dependencies: trq(sp), trk(kb//4)
        dep_ok = sp in trq_done and (kb // 4) in trk_done
        # ACT backlog: exp of chunk ci-2 must be underway before PE would
        # stall on the 2-deep psS pool; prefer chunk when ACT is hungry
        act_hungry = t['ACT'] <= t['PE'] + _chunk_pe(kb, sp) + 800

        def emit_chunk_now():
            nonlocal ci
            chunks_emit[(pair, kb, sp)] = True
            order.append(('chunk', (pair, kb, sp)))
            note_pe_item()
            t['PE'] += _chunk_pe(kb, sp)
            t['ACT'] = max(t['ACT'], t['PE']) + _chunk_act(kb, sp)
            ci += 1

        if dep_ok and act_hungry:
            emit_chunk_now()
            continue
        r = emit_filler()
        if r is None:
            # if the only blocked filler is a DMA-gated proj, stall briefly
            # instead of burning chunks we'll need at the sp boundary
            gated = any((g, s) not in proj_done
                        and t['PE'] < x_ready[g]
                        for g in range(4) for s in range(4))
            if dep_ok and not gated:
                emit_chunk_now()
            else:
                t['PE'] += 100.0  # modeled stall
    # drain remaining PV/stage3; st3s slot into the fin lag so PE never
    # sits behind a DVE/ACT norm round-trip
    while True:
        progressed = False
        for qb in range(16):
            for pair in range(2):
                if pv_ready(pair, qb):
                    emit_pv(pair, qb)
                    progressed = True
        for sb in range(16):
            if st3_ready(sb):
                emit_st3(sb, sb >= 13)
                progressed = True
        if progressed:
            continue
        if pending_fin:
            flush_fin(force=True)
            continue
        break

    # boundary post-pass: hold the last few chunks of sp=g-1 until after
    # trk(g), so PE has score work while rope(g)/trq(g) run on DVE
    for g in (1, 2, 3):
        ti = order.index(('trk', (g,)))
        movable = [i for i, (k, a) in enumerate(order[:ti])
                   if k == 'chunk' and a[2] == g - 1][-3:]
        moved = [order[i] for i in movable]
        for i in reversed(movable):
            del order[i]
        ti = order.index(('trk', (g,)))
        order[ti + 1:ti + 1] = moved

    # sanity: every pvm still follows all the chunks it reads
    seen = set()
    for k, a in order:
        if k == 'chunk':
            seen.add((a[0], a[1], a[2]))
        elif k == 'pvm':
            pair, qb = a[0], a[1]
            assert all((pair, kb, qb // 4) in seen for kb in range(qb + 1)), \
                f"post-pass broke pv dep {a}"
    return order


def _build_module():
    from contextlib import ExitStack

    import concourse.bacc as bacc
    import concourse.mybir as mybir
    import concourse.tile as tile
    from concourse.alu_op_type import AluOpType

    f32 = mybir.dt.float32
    bf16 = mybir.dt.bfloat16
    Exp = mybir.ActivationFunctionType.Exp
    Copy = mybir.ActivationFunctionType.Copy
    mult, add, sub = AluOpType.mult, AluOpType.add, AluOpType.subtract

    nc = bacc.Bacc("TRN2", target_bir_lowering=False, debug=False,
                   num_devices=N_CORES)

    xT_d = nc.dram_tensor("xT", (D, S), bf16, kind="ExternalInput").ap()
    wcat_d = nc.dram_tensor("wcatT", (D, QKV), bf16, kind="ExternalInput").ap()
    woT_d = nc.dram_tensor("woT", (2 * 128, D), bf16, kind="ExternalInput").ap()
    ctk_d = nc.dram_tensor("ctk", (128, NSB * 32), bf16, kind="ExternalInput").ap()
    stk_d = nc.dram_tensor("stk", (128, NSB * 32), bf16, kind="ExternalInput").ap()
    mneg_d = nc.dram_tensor("maskneg", (128, 128), bf16, kind="ExternalInput").ap()
    idn_d = nc.dram_tensor("ident", (128, 128), bf16, kind="ExternalInput").ap()
    y_d = nc.dram_tensor("y", (S, D), bf16, kind="ExternalOutput").ap()

    order = _schedule()

    with tile.TileContext(nc) as tc:
        with ExitStack() as ctx:
            persist = ctx.enter_context(tc.tile_pool(name="persist", bufs=1))
            woT = persist.tile([128, 2 * D], bf16)       # 4 KB
            mneg = persist.tile([128, 128], bf16)
            idn = persist.tile([128, 128], bf16)
            qT = persist.tile([128, 2 * S], bf16)        # 8 KB
            kT2 = persist.tile([128, S], bf16)           # 4 KB
            vb = persist.tile([128, NSB * 65], bf16)     # ~2 KB (v + ones col)
            eS = persist.tile([128, EBUF_SIZE], bf16, name="eS")
            dummy = persist.tile([1, 8], f32)
            attnT = [persist.tile([128, S], bf16, name="attnT0"),
                     persist.tile([128, S], bf16, name="attnT1")]

            recp = ctx.enter_context(tc.tile_pool(name="recip", bufs=2))
            anat = ctx.enter_context(tc.tile_pool(name="anat", bufs=2))

            # PSUM budget (banks): ctx psS 2x2 + pvp 2x1 = 6; s1 psq 1 +
            # tp 1 -> peak 8; post-close 6 + ypp 1x2 = 8
            psS = ctx.enter_context(
                tc.tile_pool(name="psS", bufs=2, space="PSUM"))
            pvp = ctx.enter_context(
                tc.tile_pool(name="pv", bufs=2, space="PSUM"))

            # stage-1 only pools/tiles: freed at the first stage3 emission
            # (stage1 is fully emitted by then), making room for ystage
            s1 = ExitStack()
            s1p = s1.enter_context(tc.tile_pool(name="s1p", bufs=1))
            wcat = s1p.tile([128, NDC * QKV], bf16)  # 6 KB
            ctk = s1p.tile([128, NSB * 32], bf16)
            stk = s1p.tile([128, NSB * 32], bf16)
            xtp = s1.enter_context(tc.tile_pool(name="xtp", bufs=2))
            qkvp = s1.enter_context(tc.tile_pool(name="qkvp", bufs=2))
            tmp = s1.enter_context(tc.tile_pool(name="ropetmp", bufs=1))
            psq = s1.enter_context(
                tc.tile_pool(name="psqkv", bufs=1, space="PSUM"))
            tp = s1.enter_context(
                tc.tile_pool(name="tp", bufs=1, space="PSUM"))
            yst = None  # opened after s1 closes
            ypp = None

            # ---- DMA prelude: ordered for the group-0 critical chain ----
            xr = xT_d.rearrange("(dc p) s -> p dc s", dc=NDC)
            xtg = {}

            def x_strip(g, lo, hi):
                nc.sync.dma_start(
                    xtg[g][:].rearrange("p (dc s) -> p dc s", dc=NDC)[
                        :, :, lo:hi],
                    xr[:, :, g * 512 + lo:g * 512 + hi])

            for g in range(4):
                xtg[g] = xtp.tile([128, NDC * 512], bf16, tag="xtg",
                                  name=f"xtg{g}") if g < 2 else None

            def x_dcs(g, dlo, dhi):
                nc.sync.dma_start(
                    xtg[g][:, dlo * 512:dhi * 512].rearrange(
                        "p (dc s) -> p dc s", dc=dhi - dlo),
                    xr[:, dlo:dhi, g * 512:(g + 1) * 512])

            def wcat_dcs(dlo, dhi):
                nc.sync.dma_start(
                    wcat[:, dlo * QKV:dhi * QKV].rearrange(
                        "p (dc q) -> p dc q", dc=dhi - dlo),
                    wcat_d.rearrange("(dc p) q -> p dc q", dc=NDC)[:, dlo:dhi])

            def x_dcs_cols(g, dlo, dhi, clo, chi):
                nc.sync.dma_start(
                    xtg[g][:, dlo * 512:dhi * 512].rearrange(
                        "p (dc s) -> p dc s", dc=dhi - dlo)[:, :, clo:chi],
                    xr[:, dlo:dhi, g * 512 + clo:g * 512 + chi])

            # interleave wcat/x dc-halves so the g0 proj chains can start
            # accumulating dc 0-3 while dc 4-7 still stream in; the first
            # x piece covers only s-blocks 0-1 so chain 0a starts earliest
            wcat_dcs(0, 4)
            x_dcs_cols(0, 0, 4, 0, 256)
            wcat_dcs(4, 8)
            x_dcs_cols(0, 0, 4, 256, 512)
            x_dcs(0, 4, 8)
            nc.sync.dma_start(ctk[:], ctk_d[:])
            nc.sync.dma_start(stk[:], stk_d[:])
            nc.sync.dma_start(idn[:], idn_d[:])
            nc.sync.dma_start(mneg[:], mneg_d[:])
            x_strip(1, 0, 512)
            nc.gpsimd.memset(vb[:], 1.0)
            # warm the ACT exp table while DMAs run
            nc.gpsimd.memset(dummy[:], 0.0)
            nc.scalar.activation(dummy[:], dummy[:], Exp)

            late_dma_done = {}

            def late_dmas(g):
                """x for group g+2 plus per-group tables, issued as stage1(g)
                finishes so they never gate the g0/g1 critical path."""
                if g in late_dma_done:
                    return
                late_dma_done[g] = True
                if g == 0:
                    xtg[2] = xtp.tile([128, NDC * 512], bf16, tag="xtg",
                                      name="xtg2")
                    x_strip(2, 0, 512)
                elif g == 1:
                    xtg[3] = xtp.tile([128, NDC * 512], bf16, tag="xtg",
                                      name="xtg3")
                    x_strip(3, 0, 512)
                elif g == 2:
                    nc.sync.dma_start(
                        woT[:].rearrange("p (c d) -> p c d", c=2),
                        woT_d.rearrange("(c p) d -> p c d", c=2))

            # ---- emit functions --------------------------------------------
            # qkv group layout: col = sbl*320 + h*64 + half*32 + j  (q)
            #                   col = sbl*320 + 256 + half*32 + j   (k)
            qkv_t = {}
            ps_t = {}

            def proj_mms(g, sbl, ps, dlo, dhi):
                for dc in range(dlo, dhi):
                    nc.tensor.matmul(
                        ps[:],
                        lhsT=xtg[g][:, dc * 512 + sbl * 128:
                                    dc * 512 + (sbl + 1) * 128],
                        rhs=wcat[:, dc * QKV:(dc + 1) * QKV],
                        start=(dc == 0), stop=(dc == NDC - 1))

            def proj_copies(g, sbl):
                qkv = qkv_t[g]
                sb = 4 * g + sbl
                ps = ps_t[(g, sbl)]
                if g == 0 and sbl % 2 == 0:
                    # g0 only: ACT is still idle pre-exp; later groups keep
                    # ACT clear for the exp stream
                    nc.scalar.copy(
                        qkv[:, sbl * 320: sbl * 320 + 320], ps[:, 0:320])
                    nc.scalar.copy(
                        vb[:, sb * 65: sb * 65 + 64], ps[:, 320:384])
                else:
                    nc.vector.tensor_copy(
                        qkv[:, sbl * 320: sbl * 320 + 320], ps[:, 0:320])
                    nc.vector.tensor_copy(
                        vb[:, sb * 65: sb * 65 + 64], ps[:, 320:384])
                if sbl == 3:
                    late_dmas(g)

            def alloc_ps(g, sbl):
                if sbl == 0:
                    qkv_t[g] = qkvp.tile([128, 4 * 320], bf16, tag="qkv",
                                         name=f"qkv{g}")
                sb = 4 * g + sbl
                # g0 kickoff: sbl 0/2/3 use psS slots, sbl1 psq, so the
                # split chains can stay open concurrently
                if g == 0 and sbl != 1:
                    ps = psS.tile([128, QKV], f32, tag="pp", name=f"ps{sb}")
                else:
                    ps = psq.tile([128, QKV], f32, tag="ps", name=f"ps{sb}")
                ps_t[(g, sbl)] = ps
                return ps

            def emit_proj_a(g, sbl):
                ps = alloc_ps(g, sbl)
                proj_mms(g, sbl, ps, 0, 4)

            def emit_proj_b(g, sbl):
                proj_mms(g, sbl, ps_t[(g, sbl)], 4, NDC)
                proj_copies(g, sbl)

            def emit_proj(g, sbl):
                ps = alloc_ps(g, sbl)
                proj_mms(g, sbl, ps, 0, NDC)
                proj_copies(g, sbl)

            def emit_rope(g):
                qkv = qkv_t[g]
                g4 = qkv[:].rearrange("p (sbl x) -> p sbl x", sbl=4)
                qg = g4[:, :, 0:256].rearrange("p sbl (h c) -> p sbl h c",
                                               c=64)
                qr, qi = qg[:, :, :, 0:32], qg[:, :, :, 32:64]
                kg = g4[:, :, 256:320]
                kr, ki = kg[:, :, 0:32], kg[:, :, 32:64]
                # q-rope tables are the k tables broadcast across the 4
                # query heads (stride-0 head dim)
                ct = ctk[:, g * 128:(g + 1) * 128].rearrange(
                    "p (sbl o j) -> p sbl o j", sbl=4, o=1).broadcast_to(
                    (128, 4, REP, 32))
                st = stk[:, g * 128:(g + 1) * 128].rearrange(
                    "p (sbl o j) -> p sbl o j", sbl=4, o=1).broadcast_to(
                    (128, 4, REP, 32))
                ctks = ctk[:, g * 128:(g + 1) * 128].rearrange(
                    "p (sbl j) -> p sbl j", sbl=4)
                stks = stk[:, g * 128:(g + 1) * 128].rearrange(
                    "p (sbl j) -> p sbl j", sbl=4)
                tA = tmp.tile([128, 512], bf16, tag="tA", name=f"tA{g}")
                tB = tmp.tile([128, 512], bf16, tag="tB", name=f"tB{g}")
                tC = tmp.tile([128, 512], bf16, tag="tC", name=f"tC{g}")
                tD = tmp.tile([128, 512], bf16, tag="tD", name=f"tD{g}")
                r3 = lambda t: t[:].rearrange("p (sbl h j) -> p sbl h j",
                                              sbl=4, h=REP)
                nc.vector.tensor_tensor(r3(tA), qr, ct, mult)
                nc.vector.tensor_tensor(r3(tB), qi, st, mult)
                nc.vector.tensor_tensor(r3(tC), qr, st, mult)
                nc.vector.tensor_tensor(r3(tD), qi, ct, mult)
                nc.vector.tensor_tensor(qr, r3(tA), r3(tB), sub)
                nc.vector.tensor_tensor(qi, r3(tC), r3(tD), add)
                tE = tmp.tile([128, 128], bf16, tag="tE", name=f"tE{g}")
                tF = tmp.tile([128, 128], bf16, tag="tF", name=f"tF{g}")
                tG = tmp.tile([128, 128], bf16, tag="tG", name=f"tG{g}")
                tH = tmp.tile([128, 128], bf16, tag="tH", name=f"tH{g}")
                r2 = lambda t: t[:].rearrange("p (sbl j) -> p sbl j", sbl=4)
                nc.vector.tensor_tensor(r2(tE), kr, ctks, mult)
                nc.vector.tensor_tensor(r2(tF), ki, stks, mult)
                nc.vector.tensor_tensor(r2(tG), kr, stks, mult)
                nc.vector.tensor_tensor(r2(tH), ki, ctks, mult)
                nc.vector.tensor_tensor(kr, r2(tE), r2(tF), sub)
                nc.vector.tensor_tensor(ki, r2(tG), r2(tH), add)

            def emit_trq(g, half):
                qkv = qkv_t[g]
                pt = tp.tile([128, 512], bf16, tag="ptq",
                             name=f"ptq{g}_{half}")
                for li, (sbl, hb) in enumerate(
                        [(2 * half, 0), (2 * half, 1),
                         (2 * half + 1, 0), (2 * half + 1, 1)]):
                    src = qkv[:, sbl * 320 + hb * 128:
                              sbl * 320 + hb * 128 + 128]
                    nc.tensor.transpose(
                        pt[:, li * 128:(li + 1) * 128], src, idn[:])
                dst = qT[:].rearrange(
                    "p (hb sb c) -> p sb hb c", hb=2, sb=NSB)[
                    :, 4 * g + 2 * half: 4 * g + 2 * half + 2, :, :]
                nc.vector.tensor_copy(
                    dst, pt[:].rearrange("p (sb hb c) -> p sb hb c",
                                         sb=2, hb=2))

            def emit_trk(g):
                qkv = qkv_t[g]
                ptk = tp.tile([64, 512], bf16, tag="ptq", name=f"ptk{g}")
                for sbl in range(4):
                    nc.tensor.transpose(
                        ptk[:, sbl * 128:(sbl + 1) * 128],
                        qkv[:, sbl * 320 + 256: sbl * 320 + 320],
                        idn[:])
                nc.vector.tensor_copy(
                    kT2[0:64, g * 512:(g + 1) * 512], ptk[:])
                nc.vector.tensor_copy(
                    kT2[64:128, g * 512:(g + 1) * 512], ptk[:])

            def emit_chunk(pair, kb, sp):
                w = _w_of(kb, sp)
                qlo = max(sp * 512, 128 * kb)
                qhi = (sp + 1) * 512
                diag = 128 * kb == qlo
                pp = psS.tile([128, 1024], f32, tag="pp",
                              name=f"pp{pair}_{kb}_{sp}")
                for i in range(2):
                    # head i region starts at i*512: PSUM zero-regions are
                    # 2KB-bank granular, so each group must be bank-aligned
                    nc.tensor.matmul(
                        pp[:, i * 512: i * 512 + w],
                        lhsT=kT2[i * 64:(i + 1) * 64,
                                 kb * 128:(kb + 1) * 128],
                        rhs=qT[i * 64:(i + 1) * 64,
                               pair * S + qlo: pair * S + qhi],
                        start=True, stop=True)
                off = SPOFF[sp][(pair, kb)]
                src = pp[:].rearrange("p (h c) -> p h c", h=2)[:, :, 0:w]
                dst = eS[:, off: off + 2 * w].rearrange(
                    "p (h c) -> p h c", h=2)
                nc.scalar.activation(dst, src, Exp, scale=0.125)
                if diag:
                    # causal mask: zero exp'd upper-triangle entries of the
                    # diagonal 128x128 block on the otherwise-idle Pool
                    # engine (mneg holds the 0/1 lower-triangle mask)
                    dg = eS[:, off: off + 2 * w].rearrange(
                        "p (h c) -> p h c", h=2)[:, :, 0:128]
                    mb = mneg[:].rearrange(
                        "p (o c) -> p o c", o=1).broadcast_to((128, 2, 128))
                    nc.gpsimd.tensor_tensor(dg, dg, mb, mult)

            an_t = {}

            def emit_pv_mm(pair, qb, late=False):
                sp = qb // 4
                # post-chunk-stream PVs take the freed score slots so their
                # accumulators don't WAR-serialize on each other's norms
                pool, ptag = (psS, "pp") if late else (pvp, "po")
                po = pool.tile([128, 130], f32, tag=ptag,
                               name=f"po{pair}_{qb}")
                # serialize heads: interleaving start=True groups in one
                # bank wipes the other group's pending-zero bytes
                for i in range(2):
                    for kb in range(qb + 1):
                        w = _w_of(kb, sp)
                        qlo = max(sp * 512, 128 * kb)
                        col0 = SPOFF[sp][(pair, kb)] + qb * 128 - qlo
                        nc.tensor.matmul(
                            po[:, i * 65:(i + 1) * 65],
                            lhsT=eS[:, col0 + i * w:
                                    col0 + i * w + 128],
                            rhs=vb[:, kb * 65: kb * 65 + 65],
                            start=(kb == 0), stop=(kb == qb))
                rc = recp.tile([128, 2], f32, tag="rc",
                               name=f"rc{pair}_{qb}")
                nc.vector.reciprocal(
                    rc[:], po[:].rearrange("p (h c) -> p h c",
                                           h=2)[:, :, 64])
                an = anat.tile([128, 128], bf16, tag="an",
                               name=f"an{pair}_{qb}")
                # single fused normalize: rc column broadcast over each
                # head's 64 output columns (stride-0 free dim)
                rcb = rc[:].rearrange(
                    "p (h j) -> p h j", j=1).broadcast_to((128, 2, 64))
                nc.vector.tensor_tensor(
                    an[:].rearrange("p (h j) -> p h j", h=2),
                    po[:].rearrange("p (h c) -> p h c", h=2)[:, :, 0:64],
                    rcb, mult)
                an_t[(pair, qb)] = an

            def emit_pv_fin(pair, qb):
                pt = pvp.tile([128, 128], bf16, tag="po",
                              name=f"pt{pair}_{qb}")
                an = an_t.pop((pair, qb))
                nc.tensor.transpose(pt[:], an[:], idn[:])
                if qb >= 14:
                    nc.scalar.copy(
                        attnT[pair][:, qb * 128:(qb + 1) * 128], pt[:])
                else:
                    nc.vector.tensor_copy(
                        attnT[pair][:, qb * 128:(qb + 1) * 128], pt[:])

            def emit_stage3(sb, split):
                nonlocal s1, yst, ypp
                if yst is None:
                    s1.close()
                    yst = ctx.enter_context(
                        tc.tile_pool(name="ystage", bufs=4))
                    ypp = ctx.enter_context(
                        tc.tile_pool(name="ypp", bufs=2, space="PSUM"))
                ys = yst.tile([128, D], bf16, tag="ys", name=f"ys{sb}")
                # separate half-tiles so half c2=1's matmuls don't WAR on
                # half c2=0's PSUM->SBUF copies (deps are tile-granular)
                for c2 in range(2):
                    yph = ypp.tile([128, 512], f32, tag="yp",
                                   name=f"yp{sb}_{c2}")
                    for hp in range(2):
                        nc.tensor.matmul(
                            yph[:],
                            lhsT=attnT[hp][:, sb * 128:(sb + 1) * 128],
                            rhs=woT[:, hp * D + c2 * 512:
                                    hp * D + (c2 + 1) * 512],
                            start=(hp == 0), stop=(hp == 1))
                    lo = c2 * 512
                    if split:
                        # tail: halves on alternating copy engines and DMA
                        # dispatch queues; the very last half goes out in
                        # quarters so the final copy->DMA->sem chain is short
                        if c2 == 0:
                            nc.vector.tensor_copy(
                                ys[:, lo:lo + 512], yph[:])
                            nc.sync.dma_start(
                                y_d[sb * 128:(sb + 1) * 128, lo:lo + 512],
                                ys[:, lo:lo + 512])
                        else:
                            nc.vector.tensor_copy(
                                ys[:, lo:lo + 256], yph[:, 0:256])
                            nc.sync.dma_start(
                                y_d[sb * 128:(sb + 1) * 128, lo:lo + 256],
                                ys[:, lo:lo + 256])
                            nc.scalar.copy(
                                ys[:, lo + 256:lo + 512], yph[:, 256:512])
                            nc.scalar.dma_start(
                                y_d[sb * 128:(sb + 1) * 128,
                                    lo + 256:lo + 512],
                                ys[:, lo + 256:lo + 512])
                    else:
                        nc.vector.tensor_copy(
                            ys[:, lo:lo + 512], yph[:])
                        if c2 == 1:
                            nc.sync.dma_start(
                                y_d[sb * 128:(sb + 1) * 128, :], ys[:])

            # ---- run the schedule ----
            emitters = {
                'proj': emit_proj, 'proja': emit_proj_a, 'projb': emit_proj_b,
                'rope': emit_rope, 'trq': emit_trq, 'trk': emit_trk,
                'chunk': emit_chunk, 'pvm': emit_pv_mm, 'pvf': emit_pv_fin,
                'st3': emit_stage3,
            }
            for kind, args in order:
                emitters[kind](*args)

    nc.compile()
    return nc


def _get_module():
    if "nc" not in _CACHE:
        _CACHE["nc"] = _build_module()
    return _CACHE["nc"]


def _host_tables(freqs_cos, freqs_sin):
    # ctk[p, sb*32 + j] = cos[sb*128 + p, j]; q-rope reads the same table
    # with a stride-0 head broadcast on-device
    c3 = freqs_cos.reshape(NSB, 128, 32).transpose(1, 0, 2)  # [p, sb, j]
    s3 = freqs_sin.reshape(NSB, 128, 32).transpose(1, 0, 2)
    ctk = np.ascontiguousarray(c3).reshape(128, NSB * 32)
    stk = np.ascontiguousarray(s3).reshape(128, NSB * 32)
    return ctk, stk


def make_in_maps(x, wq, wk, wv, wo, freqs_cos, freqs_sin):
    x = np.asarray(x, np.float32)
    wq = np.asarray(wq, np.float32)
    wk = np.asarray(wk, np.float32)
    wv = np.asarray(wv, np.float32)
    wo = np.asarray(wo, np.float32)
    freqs_cos = np.asarray(freqs_cos, np.float32)
    freqs_sin = np.asarray(freqs_sin, np.float32)

    # deinterleave rope pairs within each head: [r0 i0 r1 i1 ...] ->
    # [r0..r31 | i0..i31]
    idx = np.concatenate([np.arange(0, HD, 2), np.arange(1, HD, 2)])
    wq_p = wq.reshape(H, HD, D)[:, idx, :].reshape(H * HD, D)
    wk_p = wk.reshape(HKV, HD, D)[:, idx, :].reshape(HKV * HD, D)

    ctk, stk = _host_tables(freqs_cos, freqs_sin)
    # 0/1 lower-triangle multiplicative mask (k <= q keeps, k > q zeroed)
    kk, qq = np.arange(128)[:, None], np.arange(128)[None, :]
    maskneg = np.where(kk <= qq, 1.0, 0.0).astype(np.float32)
    ident = np.eye(128)

    common = {
        "ctk": ctk.astype(BF16), "stk": stk.astype(BF16),
        "maskneg": maskneg.astype(BF16), "ident": ident.astype(BF16),
    }
    xT_b = [np.ascontiguousarray(x[b].T).astype(BF16) for b in range(B)]
    in_maps = []
    for core in range(N_CORES):
        b, g = divmod(core, HKV)
        wqT = wq_p[g * 256:(g + 1) * 256].T
        wkT = wk_p[g * 64:(g + 1) * 64].T
        wvT = wv[g * 64:(g + 1) * 64].T
        wcat = np.ascontiguousarray(
            np.concatenate([wqT, wkT, wvT], axis=1)).astype(BF16)
        woTg = np.ascontiguousarray(wo[:, g * 256:(g + 1) * 256].T).astype(BF16)
        in_maps.append({"xT": xT_b[b], "wcatT": wcat, "woT": woTg, **common})
    return in_maps


def _causal_fast_path_ok(mask):
    m = np.asarray(mask)
    if m.shape != (S, S):
        return False
    upper = m[np.triu_indices(S, 1)]
    lower = m[np.tril_indices(S, 0)]
    return bool(np.all(upper <= -1e8) and np.all(lower == 0))


def _numpy_fallback(x, wq, wk, wv, wo, freqs_cos, freqs_sin, mask):
    x = np.asarray(x, np.float32)
    xq = (x.reshape(B * S, D) @ np.asarray(wq, np.float32).T).reshape(B, S, H, HD)
    xk = (x.reshape(B * S, D) @ np.asarray(wk, np.float32).T).reshape(B, S, HKV, HD)
    xv = (x.reshape(B * S, D) @ np.asarray(wv, np.float32).T).reshape(B, S, HKV, HD)

    def rope(t, nh):
        tf = t.reshape(B, S, nh, HD // 2, 2)
        tr, ti = tf[..., 0], tf[..., 1]
        c = np.asarray(freqs_cos, np.float32)[None, :, None, :]
        s = np.asarray(freqs_sin, np.float32)[None, :, None, :]
        outr = tr * c - ti * s
        outi = tr * s + ti * c
        return np.stack([outr, outi], axis=-1).reshape(B, S, nh, HD)

    xq = rope(xq, H)
    xk = rope(xk, HKV)
    xqg = xq.reshape(B, S, HKV, REP, HD)
    scores = np.einsum("bqgrd,bkgd->bgrqk", xqg, xk) / np.sqrt(np.float32(HD))
    scores = scores + np.asarray(mask, np.float32)[None, None, None, :, :]
    scores = scores - scores.max(axis=-1, keepdims=True)
    e = np.exp(scores)
    attn = e / e.sum(axis=-1, keepdims=True)
    out = np.einsum("bgrqk,bkgd->bqgrd", attn, xv).reshape(B, S, H * HD)
    return (out.reshape(B * S, H * HD) @ np.asarray(wo, np.float32)
            .T).reshape(B, S, D).astype(np.float32)


def kernel(x, wq, wk, wv, wo, freqs_cos, freqs_sin, mask):
    if not _causal_fast_path_ok(mask):
        return _numpy_fallback(x, wq, wk, wv, wo, freqs_cos, freqs_sin, mask)
    from concourse import bass_utils
    nc = _get_module()
    in_maps = make_in_maps(x, wq, wk, wv, wo, freqs_cos, freqs_sin)
    res = bass_utils.run_bass_kernel_spmd(nc, in_maps,
                                          core_ids=list(range(N_CORES)))
    y = np.zeros((B, S, D), np.float32)
    for core in range(N_CORES):
        b = core // HKV
        y[b] += res.results[core]["y"].astype(np.float32)
    return y


# revision 60
# speedup vs baseline: 1.0027x; 1.0027x over previous
"""Trainium2 Bass kernel for GQA attention (B=2, S=2048, D=1024, H=16, HKV=4).

Sharding: 8 cores = batch(2) x kv-group(4). Each core handles one batch and
one KV head group (4 query heads + 1 KV head), computes attention plus its
partial slice of the output projection (row-parallel wo); the host sums the
4 tensor-parallel partials per batch (partials are bf16, summed in f32).
No device collectives.

Per-core device kernel (matmul operands bf16, fp32 PSUM accumulation).
Work units:
  - proj(g, sbl): QKV projection for s-block 4g+sbl (8 dc-chunk matmuls into
    one PSUM accumulator, then PSUM->SBUF copy of q/k to qkv and v to vb).
  - rope(g): RoPE for group g's 4 s-blocks on DVE (weights pre-permuted so
    each head is [32 real | 32 imag]).
  - trq(g, half)/trk(g): PE-transpose q -> qT and k -> kT2 (k duplicated to
    partitions 64-127 for two-head row tiling).
  - chunk(pair, kb, sp): scores S^T = kT.T @ qT for one k-block over q in
    [512*sp, 512*(sp+1)), both heads of a pair in one [128, 2w] PSUM tile;
    diagonal blocks masked by accumulating identity @ (-1e9 upper-triangle);
    one exp per chunk on ACT (scale=1/8 folded; no max-subtraction).
  - pv(pair, qb): po[q=128, 2*65] accumulates eS[k,q].T @ v[k, 65] over
    kb <= qb with a ones column collecting the softmax denominator, then
    DVE reciprocal + tensor_scalar normalize, PE-transpose to attnT.
  - stage3(sb): y = attnT.T @ woT for s-block sb, PSUM->SBUF copy, DMA out.

The emission order is produced by a greedy static scheduler that models
per-engine clocks (PE/ACT/DVE/DMA) and interleaves score chunks with all
other PE work so the ACT exp stream (the ~60us serial floor) runs
continuously from ~8us on, while PE (~84us of matmul work) never waits on
exp except through the 2-deep score PSUM pool.
"""

import numpy as np
import ml_dtypes

B, S, D = 2, 2048, 1024
H, HKV, HD = 16, 4, 64
REP = H // HKV  # 4 query heads per kv head
N_CORES = 8
NSB = S // 128  # 16 s-blocks
NDC = D // 128  # 8 d-chunks
QKV = REP * HD + 2 * HD  # 384 projected dims per core
NSP = 4  # q subpasses of 512
BF16 = ml_dtypes.bfloat16

_CACHE = {}


def _w_of(kb, sp):
    """q-width of the (kb, sp) score chunk (per head)."""
    return (sp + 1) * 512 - max(sp * 512, 128 * kb)


def _spoff():
    """eSb slot offsets: off[sp][(pair, kb)] in the single resident buffer."""
    offs = []
    cum = 0
    for sp in range(NSP):
        off = {}
        for pair in range(2):
            for kb in range(4 * sp + 4):
                off[(pair, kb)] = cum
                cum += 2 * _w_of(kb, sp)
        offs.append(off)
    return offs, cum


SPOFF, EBUF_SIZE = _spoff()

# ---- static greedy schedule ------------------------------------------------
# Modeled per-instruction costs (ns), matching the TRN2 cost model closely
# enough to order emissions well.
PEC = 0.4166


def _chunk_pe(kb, sp):
    w = _w_of(kb, sp)
    diag = 128 * kb == max(sp * 512, 128 * kb)
    return 2 * w * PEC + (53 if diag else 0)


def _chunk_act(kb, sp):
    return 2 * _w_of(kb, sp) * 0.833 + 190


def _schedule():
    """Return the emission order as a list of ('kind', args) tuples."""
    order = []
    # modeled clocks
    t = {'PE': 0.0, 'ACT': 2500.0, 'DVE': 0.0}
    # DMA availability (modeled from the serialized prelude transfer queue):
    # emitting a proj before its x strip lands would stall every later PE op
    # behind it in program order, so gate hard on these
    x_ready = {0: 4500.0, 1: 10000.0, 2: 13000.0, 3: 16000.0}

    proj_done = {}     # (g, sbl) -> emitted
    rope_done = set()
    trq_done = set()   # g
    trk_done = set()   # g
    chunks_emit = {}   # (pair, kb, sp) -> modeled exp end time
    pv_done = set()    # (pair, qb): FIN emitted (attnT written)
    st3_done = set()
    pending_fin = []   # pv fins waiting to be emitted one PE-item later

    def flush_fin(force=False):
        while pending_fin and (force or len(pending_fin) >= 2):
            order.append(('pvf', pending_fin.pop(0)))

    def note_pe_item():
        """Called after each PE-occupying emission: release one lagging fin
        so its transpose lands behind fresh PE work instead of a DVE wait."""
        if pending_fin:
            order.append(('pvf', pending_fin.pop(0)))

    # chunk stream order: sp ascending, kb ascending, pairs interleaved;
    # late-sp2 chunks are interleaved into early sp3 so their PVs (8-11)
    # become available as filler inside the long sp3 exp stretch
    chunk_list = []
    for sp in range(2):
        for kb in range(4 * sp + 4):
            for pair in range(2):
                chunk_list.append((pair, kb, sp))
    for kb in range(8):
        for pair in range(2):
            chunk_list.append((pair, kb, 2))
    late2 = [(pair, kb, 2) for kb in range(8, 12) for pair in range(2)]
    sp3 = [(pair, kb, 3) for kb in range(16) for pair in range(2)]
    while late2 or sp3:
        if late2:
            chunk_list.append(late2.pop(0))
        for _ in range(3):
            if sp3:
                chunk_list.append(sp3.pop(0))
    ci = 0

    def stage1_ready(g):
        return all((g, s) in proj_done for s in range(4))

    def emit_stage1_piece():
        """Emit the next stage1 piece (proj/rope/tr) if one is ready.
        Returns PE-time consumed (0 for DVE-only pieces)."""
        nonlocal order
        for g in range(4):
            for sbl in range(4):
                if (g, sbl) not in proj_done:
                    if t['PE'] < x_ready[g]:
                        return None  # inputs not there yet; don't stall PE
                    proj_done[(g, sbl)] = True
                    order.append(('proj', (g, sbl)))
                    note_pe_item()
                    t['PE'] += 8 * 384 * PEC
                    return 8 * 384 * PEC
            if g not in rope_done:
                rope_done.add(g)
                order.append(('rope', (g,)))
                t['DVE'] += 12 * 320
                return 0.0
            if g not in trq_done:
                trq_done.add(g)
                trk_done.add(g)
                order.append(('trq', (g, 0)))
                order.append(('trq', (g, 1)))
                order.append(('trk', (g,)))
                t['PE'] += 12 * 53
                return 12 * 53
        return None

    def pv_ready(pair, qb):
        sp = qb // 4
        return ((pair, qb) not in pv_done
                and all((pair, kb, sp) in chunks_emit
                        for kb in range(qb + 1)))

    def emit_pv(pair, qb):
        order.append(('pvm', (pair, qb, ci >= len(chunk_list))))
        note_pe_item()
        pending_fin.append((pair, qb))
        pv_done.add((pair, qb))
        dt = 2 * (qb + 1) * 65 * PEC + 53
        t['PE'] += dt
        t['DVE'] += 700
        return dt

    def st3_ready(sb):
        return (sb not in st3_done
                and (0, sb) in pv_done and (1, sb) in pv_done
                and (0, sb) not in pending_fin
                and (1, sb) not in pending_fin)

    def emit_st3(sb, split):
        order.append(('st3', (sb, split)))
        note_pe_item()
        st3_done.add(sb)
        t['PE'] += 4 * 512 * PEC
        t['DVE'] += 660
        return 4 * 512 * PEC

    # ACT-surplus of each chunk: filler PE-time the schedule must supply
    # alongside it for PE not to block on the 2-deep psS pool
    surplus = [max(0.0, _chunk_act(kb, sp) - _chunk_pe(kb, sp))
               for (_, kb, sp) in chunk_list]
    suffix_surplus = [0.0] * (len(chunk_list) + 1)
    for i in range(len(chunk_list) - 1, -1, -1):
        suffix_surplus[i] = suffix_surplus[i + 1] + surplus[i]

    def flex_left():
        """PE-time of unemitted PV + stage3 work."""
        tot = 0.0
        for qb in range(16):
            for pair in range(2):
                if (pair, qb) not in pv_done:
                    tot += 2 * (qb + 1) * 65 * PEC + 53
        tot += (16 - len(st3_done)) * 4 * 512 * PEC
        return tot

    def emit_filler():
        """Pick one filler item; returns PE ns consumed or None.
        PV/stage3 are hoarded unless we hold more than future chunks need."""
        r = emit_stage1_piece()
        if r is not None:
            return r
        if flex_left() <= suffix_surplus[ci]:
            return None  # reserve PV/stage3 for later ACT-bound stretches
        # PV with the lowest qb ready
        for qb in range(16):
            for pair in range(2):
                if (pair, qb) not in pv_done and pv_ready(pair, qb):
                    return emit_pv(pair, qb)
        # stage3 for blocks whose both PVs are done (keep one-block lag)
        ready3 = [sb for sb in range(16) if st3_ready(sb)]
        if len(ready3) >= 2 or (ready3 and ci >= len(chunk_list)):
            return emit_st3(ready3[0], False)
        return None

    # kick off: g0 proj with dc0-3/dc4-7 split chains over 3 accumulators
    # (sbl 0-2) so PE starts as soon as the first x/wcat halves land; sbl3
    # runs whole once everything is resident
    for sbl in range(3):
        order.append(('proja', (0, sbl)))
    for sbl in range(3):
        order.append(('projb', (0, sbl)))
    order.append(('proj', (0, 3)))
    for sbl in range(4):
        proj_done[(0, sbl)] = True
    t['PE'] = x_ready[0] + 4 * 8 * 384 * PEC
    while not (stage1_ready(0) and 0 in trq_done):
        r = emit_stage1_piece()
        if r is None:
            t['PE'] += 100.0  # modeled wait for DMA

    while ci < len(chunk_list):
        pair, kb, sp = chunk_list[ci]
        # dependencies: trq(sp), trk(kb//4)
        dep_ok = sp in trq_done and (kb // 4) in trk_done
        # ACT backlog: exp of chunk ci-2 must be underway before PE would
        # stall on the 2-deep psS pool; prefer chunk when ACT is hungry
        act_hungry = t['ACT'] <= t['PE'] + _chunk_pe(kb, sp) + 800

        def emit_chunk_now():
            nonlocal ci
            chunks_emit[(pair, kb, sp)] = True
            order.append(('chunk', (pair, kb, sp)))
            note_pe_item()
            t['PE'] += _chunk_pe(kb, sp)
            t['ACT'] = max(t['ACT'], t['PE']) + _chunk_act(kb, sp)
            ci += 1

        if dep_ok and act_hungry:
            emit_chunk_now()
            continue
        r = emit_filler()
        if r is None:
            # if the only blocked filler is a DMA-gated proj, stall briefly
            # instead of burning chunks we'll need at the sp boundary
            gated = any((g, s) not in proj_done
                        and t['PE'] < x_ready[g]
                        for g in range(4) for s in range(4))
            if dep_ok and not gated:
                emit_chunk_now()
            else:
                t['PE'] += 100.0  # modeled stall
    # drain remaining PV/stage3; st3s slot into the fin lag so PE never
    # sits behind a DVE/ACT norm round-trip
    while True:
        progressed = False
        for qb in range(16):
            for pair in range(2):
                if pv_ready(pair, qb):
                    emit_pv(pair, qb)
                    progressed = True
        for sb in range(16):
            if st3_ready(sb):
                emit_st3(sb, sb >= 13)
                progressed = True
        if progressed:
            continue
        if pending_fin:
            flush_fin(force=True)
            continue
        break

    # boundary post-pass: hold the last two chunks of sp=g-1 until after
    # trk(g), so PE has score work while rope(g)/trq(g) run on DVE
    for g in (1, 2):
        ti = order.index(('trk', (g,)))
        movable = [i for i, (k, a) in enumerate(order[:ti])
                   if k == 'chunk' and a[2] == g - 1][-2:]
        moved = [order[i] for i in movable]
        for i in reversed(movable):
            del order[i]
        ti = order.index(('trk', (g,)))
        order[ti + 1:ti + 1] = moved

    # sanity: every pvm still follows all the chunks it reads
    seen = set()
    for k, a in order:
        if k == 'chunk':
            seen.add((a[0], a[1], a[2]))
        elif k == 'pvm':
            pair, qb = a[0], a[1]
            assert all((pair, kb, qb // 4) in seen for kb in range(qb + 1)), \
                f"post-pass broke pv dep {a}"
    return order


def _build_module():
    from contextlib import ExitStack

    import concourse.bacc as bacc
    import concourse.mybir as mybir
    import concourse.tile as tile
    from concourse.alu_op_type import AluOpType

    f32 = mybir.dt.float32
    bf16 = mybir.dt.bfloat16
    Exp = mybir.ActivationFunctionType.Exp
    Copy = mybir.ActivationFunctionType.Copy
    mult, add, sub = AluOpType.mult, AluOpType.add, AluOpType.subtract

    nc = bacc.Bacc("TRN2", target_bir_lowering=False, debug=False,
                   num_devices=N_CORES)

    xT_d = nc.dram_tensor("xT", (D, S), bf16, kind="ExternalInput").ap()
    wcat_d = nc.dram_tensor("wcatT", (D, QKV), bf16, kind="ExternalInput").ap()
    woT_d = nc.dram_tensor("woT", (2 * 128, D), bf16, kind="ExternalInput").ap()
    ctk_d = nc.dram_tensor("ctk", (128, NSB * 32), bf16, kind="ExternalInput").ap()
    stk_d = nc.dram_tensor("stk", (128, NSB * 32), bf16, kind="ExternalInput").ap()
    mneg_d = nc.dram_tensor("maskneg", (128, 128), bf16, kind="ExternalInput").ap()
    idn_d = nc.dram_tensor("ident", (128, 128), bf16, kind="ExternalInput").ap()
    y_d = nc.dram_tensor("y", (S, D), bf16, kind="ExternalOutput").ap()

    order = _schedule()

    with tile.TileContext(nc) as tc:
        with ExitStack() as ctx:
            persist = ctx.enter_context(tc.tile_pool(name="persist", bufs=1))
            woT = persist.tile([128, 2 * D], bf16)       # 4 KB
            mneg = persist.tile([128, 128], bf16)
            idn = persist.tile([128, 128], bf16)
            qT = persist.tile([128, 2 * S], bf16)        # 8 KB
            kT2 = persist.tile([128, S], bf16)           # 4 KB
            vb = persist.tile([128, NSB * 65], bf16)     # ~2 KB (v + ones col)
            eS = persist.tile([128, EBUF_SIZE], bf16, name="eS")
            dummy = persist.tile([1, 8], f32)
            attnT = [persist.tile([128, S], bf16, name="attnT0"),
                     persist.tile([128, S], bf16, name="attnT1")]

            recp = ctx.enter_context(tc.tile_pool(name="recip", bufs=2))
            anat = ctx.enter_context(tc.tile_pool(name="anat", bufs=2))

            # PSUM budget (banks): ctx psS 2x2 + pvp 2x1 = 6; s1 psq 1 +
            # tp 1 -> peak 8; post-close 6 + ypp 1x2 = 8
            psS = ctx.enter_context(
                tc.tile_pool(name="psS", bufs=2, space="PSUM"))
            pvp = ctx.enter_context(
                tc.tile_pool(name="pv", bufs=2, space="PSUM"))

            # stage-1 only pools/tiles: freed at the first stage3 emission
            # (stage1 is fully emitted by then), making room for ystage
            s1 = ExitStack()
            s1p = s1.enter_context(tc.tile_pool(name="s1p", bufs=1))
            wcat = s1p.tile([128, NDC * QKV], bf16)  # 6 KB
            ctk = s1p.tile([128, NSB * 32], bf16)
            stk = s1p.tile([128, NSB * 32], bf16)
            xtp = s1.enter_context(tc.tile_pool(name="xtp", bufs=2))
            qkvp = s1.enter_context(tc.tile_pool(name="qkvp", bufs=2))
            tmp = s1.enter_context(tc.tile_pool(name="ropetmp", bufs=1))
            psq = s1.enter_context(
                tc.tile_pool(name="psqkv", bufs=1, space="PSUM"))
            tp = s1.enter_context(
                tc.tile_pool(name="tp", bufs=1, space="PSUM"))
            yst = None  # opened after s1 closes
            ypp = None

            # ---- DMA prelude: ordered for the group-0 critical chain ----
            xr = xT_d.rearrange("(dc p) s -> p dc s", dc=NDC)
            xtg = {}

            def x_strip(g, lo, hi):
                nc.sync.dma_start(
                    xtg[g][:].rearrange("p (dc s) -> p dc s", dc=NDC)[
                        :, :, lo:hi],
                    xr[:, :, g * 512 + lo:g * 512 + hi])

            for g in range(4):
                xtg[g] = xtp.tile([128, NDC * 512], bf16, tag="xtg",
                                  name=f"xtg{g}") if g < 2 else None

            def x_dcs(g, dlo, dhi):
                nc.sync.dma_start(
                    xtg[g][:, dlo * 512:dhi * 512].rearrange(
                        "p (dc s) -> p dc s", dc=dhi - dlo),
                    xr[:, dlo:dhi, g * 512:(g + 1) * 512])

            def wcat_dcs(dlo, dhi):
                nc.sync.dma_start(
                    wcat[:, dlo * QKV:dhi * QKV].rearrange(
                        "p (dc q) -> p dc q", dc=dhi - dlo),
                    wcat_d.rearrange("(dc p) q -> p dc q", dc=NDC)[:, dlo:dhi])

            def x_dcs_cols(g, dlo, dhi, clo, chi):
                nc.sync.dma_start(
                    xtg[g][:, dlo * 512:dhi * 512].rearrange(
                        "p (dc s) -> p dc s", dc=dhi - dlo)[:, :, clo:chi],
                    xr[:, dlo:dhi, g * 512 + clo:g * 512 + chi])

            # interleave wcat/x dc-halves so the g0 proj chains can start
            # accumulating dc 0-3 while dc 4-7 still stream in; the first
            # x piece covers only s-blocks 0-1 so chain 0a starts earliest
            wcat_dcs(0, 4)
            x_dcs_cols(0, 0, 4, 0, 256)
            wcat_dcs(4, 8)
            x_dcs_cols(0, 0, 4, 256, 512)
            x_dcs(0, 4, 8)
            nc.sync.dma_start(ctk[:], ctk_d[:])
            nc.sync.dma_start(stk[:], stk_d[:])
            nc.sync.dma_start(idn[:], idn_d[:])
            nc.sync.dma_start(mneg[:], mneg_d[:])
            x_strip(1, 0, 512)
            nc.gpsimd.memset(vb[:], 1.0)
            # warm the ACT exp table while DMAs run
            nc.gpsimd.memset(dummy[:], 0.0)
            nc.scalar.activation(dummy[:], dummy[:], Exp)

            late_dma_done = {}

            def late_dmas(g):
                """x for group g+2 plus per-group tables, issued as stage1(g)
                finishes so they never gate the g0/g1 critical path."""
                if g in late_dma_done:
                    return
                late_dma_done[g] = True
                if g == 0:
                    xtg[2] = xtp.tile([128, NDC * 512], bf16, tag="xtg",
                                      name="xtg2")
                    x_strip(2, 0, 512)
                elif g == 1:
                    xtg[3] = xtp.tile([128, NDC * 512], bf16, tag="xtg",
                                      name="xtg3")
                    x_strip(3, 0, 512)
                elif g == 2:
                    nc.sync.dma_start(
                        woT[:].rearrange("p (c d) -> p c d", c=2),
                        woT_d.rearrange("(c p) d -> p c d", c=2))

            # ---- emit functions --------------------------------------------
            # qkv group layout: col = sbl*320 + h*64 + half*32 + j  (q)
            #                   col = sbl*320 + 256 + half*32 + j   (k)
            qkv_t = {}
            ps_t = {}

            def proj_mms(g, sbl, ps, dlo, dhi):
                for dc in range(dlo, dhi):
                    nc.tensor.matmul(
                        ps[:],
                        lhsT=xtg[g][:, dc * 512 + sbl * 128:
                                    dc * 512 + (sbl + 1) * 128],
                        rhs=wcat[:, dc * QKV:(dc + 1) * QKV],
                        start=(dc == 0), stop=(dc == NDC - 1))

            def proj_copies(g, sbl):
                qkv = qkv_t[g]
                sb = 4 * g + sbl
                ps = ps_t[(g, sbl)]
                if g == 0 and sbl % 2 == 0:
                    # g0 only: ACT is still idle pre-exp; later groups keep
                    # ACT clear for the exp stream
                    nc.scalar.copy(
                        qkv[:, sbl * 320: sbl * 320 + 320], ps[:, 0:320])
                    nc.scalar.copy(
                        vb[:, sb * 65: sb * 65 + 64], ps[:, 320:384])
                else:
                    nc.vector.tensor_copy(
                        qkv[:, sbl * 320: sbl * 320 + 320], ps[:, 0:320])
                    nc.vector.tensor_copy(
                        vb[:, sb * 65: sb * 65 + 64], ps[:, 320:384])
                if sbl == 3:
                    late_dmas(g)

            def alloc_ps(g, sbl):
                if sbl == 0:
                    qkv_t[g] = qkvp.tile([128, 4 * 320], bf16, tag="qkv",
                                         name=f"qkv{g}")
                sb = 4 * g + sbl
                # g0 kickoff: sbl 0/2/3 use psS slots, sbl1 psq, so the
                # split chains can stay open concurrently
                if g == 0 and sbl != 1:
                    ps = psS.tile([128, QKV], f32, tag="pp", name=f"ps{sb}")
                else:
                    ps = psq.tile([128, QKV], f32, tag="ps", name=f"ps{sb}")
                ps_t[(g, sbl)] = ps
                return ps

            def emit_proj_a(g, sbl):
                ps = alloc_ps(g, sbl)
                proj_mms(g, sbl, ps, 0, 4)

            def emit_proj_b(g, sbl):
                proj_mms(g, sbl, ps_t[(g, sbl)], 4, NDC)
                proj_copies(g, sbl)

            def emit_proj(g, sbl):
                ps = alloc_ps(g, sbl)
                proj_mms(g, sbl, ps, 0, NDC)
                proj_copies(g, sbl)

            def emit_rope(g):
                qkv = qkv_t[g]
                g4 = qkv[:].rearrange("p (sbl x) -> p sbl x", sbl=4)
                qg = g4[:, :, 0:256].rearrange("p sbl (h c) -> p sbl h c",
                                               c=64)
                qr, qi = qg[:, :, :, 0:32], qg[:, :, :, 32:64]
                kg = g4[:, :, 256:320]
                kr, ki = kg[:, :, 0:32], kg[:, :, 32:64]
                # q-rope tables are the k tables broadcast across the 4
                # query heads (stride-0 head dim)
                ct = ctk[:, g * 128:(g + 1) * 128].rearrange(
                    "p (sbl o j) -> p sbl o j", sbl=4, o=1).broadcast_to(
                    (128, 4, REP, 32))
                st = stk[:, g * 128:(g + 1) * 128].rearrange(
                    "p (sbl o j) -> p sbl o j", sbl=4, o=1).broadcast_to(
                    (128, 4, REP, 32))
                ctks = ctk[:, g * 128:(g + 1) * 128].rearrange(
                    "p (sbl j) -> p sbl j", sbl=4)
                stks = stk[:, g * 128:(g + 1) * 128].rearrange(
                    "p (sbl j) -> p sbl j", sbl=4)
                tA = tmp.tile([128, 512], bf16, tag="tA", name=f"tA{g}")
                tB = tmp.tile([128, 512], bf16, tag="tB", name=f"tB{g}")
                tC = tmp.tile([128, 512], bf16, tag="tC", name=f"tC{g}")
                tD = tmp.tile([128, 512], bf16, tag="tD", name=f"tD{g}")
                r3 = lambda t: t[:].rearrange("p (sbl h j) -> p sbl h j",
                                              sbl=4, h=REP)
                nc.vector.tensor_tensor(r3(tA), qr, ct, mult)
                nc.vector.tensor_tensor(r3(tB), qi, st, mult)
                nc.vector.tensor_tensor(r3(tC), qr, st, mult)
                nc.vector.tensor_tensor(r3(tD), qi, ct, mult)
                nc.vector.tensor_tensor(qr, r3(tA), r3(tB), sub)
                nc.vector.tensor_tensor(qi, r3(tC), r3(tD), add)
                tE = tmp.tile([128, 128], bf16, tag="tE", name=f"tE{g}")
                tF = tmp.tile([128, 128], bf16, tag="tF", name=f"tF{g}")
                tG = tmp.tile([128, 128], bf16, tag="tG", name=f"tG{g}")
                tH = tmp.tile([128, 128], bf16, tag="tH", name=f"tH{g}")
                r2 = lambda t: t[:].rearrange("p (sbl j) -> p sbl j", sbl=4)
                nc.vector.tensor_tensor(r2(tE), kr, ctks, mult)
                nc.vector.tensor_tensor(r2(tF), ki, stks, mult)
                nc.vector.tensor_tensor(r2(tG), kr, stks, mult)
                nc.vector.tensor_tensor(r2(tH), ki, ctks, mult)
                nc.vector.tensor_tensor(kr, r2(tE), r2(tF), sub)
                nc.vector.tensor_tensor(ki, r2(tG), r2(tH), add)

            def emit_trq(g, half):
                qkv = qkv_t[g]
                pt = tp.tile([128, 512], bf16, tag="ptq",
                             name=f"ptq{g}_{half}")
                for li, (sbl, hb) in enumerate(
                        [(2 * half, 0), (2 * half, 1),
                         (2 * half + 1, 0), (2 * half + 1, 1)]):
                    src = qkv[:, sbl * 320 + hb * 128:
                              sbl * 320 + hb * 128 + 128]
                    nc.tensor.transpose(
                        pt[:, li * 128:(li + 1) * 128], src, idn[:])
                dst = qT[:].rearrange(
                    "p (hb sb c) -> p sb hb c", hb=2, sb=NSB)[
                    :, 4 * g + 2 * half: 4 * g + 2 * half + 2, :, :]
                nc.vector.tensor_copy(
                    dst, pt[:].rearrange("p (sb hb c) -> p sb hb c",
                                         sb=2, hb=2))

            def emit_trk(g):
                qkv = qkv_t[g]
                ptk = tp.tile([64, 512], bf16, tag="ptq", name=f"ptk{g}")
                for sbl in range(4):
                    nc.tensor.transpose(
                        ptk[:, sbl * 128:(sbl + 1) * 128],
                        qkv[:, sbl * 320 + 256: sbl * 320 + 320],
                        idn[:])
                nc.vector.tensor_copy(
                    kT2[0:64, g * 512:(g + 1) * 512], ptk[:])
                nc.vector.tensor_copy(
                    kT2[64:128, g * 512:(g + 1) * 512], ptk[:])

            def emit_chunk(pair, kb, sp):
                w = _w_of(kb, sp)
                qlo = max(sp * 512, 128 * kb)
                qhi = (sp + 1) * 512
                diag = 128 * kb == qlo
                pp = psS.tile([128, 1024], f32, tag="pp",
                              name=f"pp{pair}_{kb}_{sp}")
                for i in range(2):
                    # head i region starts at i*512: PSUM zero-regions are
                    # 2KB-bank granular, so each group must be bank-aligned
                    nc.tensor.matmul(
                        pp[:, i * 512: i * 512 + w],
                        lhsT=kT2[i * 64:(i + 1) * 64,
                                 kb * 128:(kb + 1) * 128],
                        rhs=qT[i * 64:(i + 1) * 64,
                               pair * S + qlo: pair * S + qhi],
                        start=True, stop=True)
                off = SPOFF[sp][(pair, kb)]
                src = pp[:].rearrange("p (h c) -> p h c", h=2)[:, :, 0:w]
                dst = eS[:, off: off + 2 * w].rearrange(
                    "p (h c) -> p h c", h=2)
                nc.scalar.activation(dst, src, Exp, scale=0.125)
                if diag:
                    # causal mask: zero exp'd upper-triangle entries of the
                    # diagonal 128x128 block on the otherwise-idle Pool
                    # engine (mneg holds the 0/1 lower-triangle mask)
                    dg = eS[:, off: off + 2 * w].rearrange(
                        "p (h c) -> p h c", h=2)[:, :, 0:128]
                    mb = mneg[:].rearrange(
                        "p (o c) -> p o c", o=1).broadcast_to((128, 2, 128))
                    nc.gpsimd.tensor_tensor(dg, dg, mb, mult)

            an_t = {}

            def emit_pv_mm(pair, qb, late=False):
                sp = qb // 4
                # post-chunk-stream PVs take the freed score slots so their
                # accumulators don't WAR-serialize on each other's norms
                pool, ptag = (psS, "pp") if late else (pvp, "po")
                po = pool.tile([128, 130], f32, tag=ptag,
                               name=f"po{pair}_{qb}")
                # serialize heads: interleaving start=True groups in one
                # bank wipes the other group's pending-zero bytes
                for i in range(2):
                    for kb in range(qb + 1):
                        w = _w_of(kb, sp)
                        qlo = max(sp * 512, 128 * kb)
                        col0 = SPOFF[sp][(pair, kb)] + qb * 128 - qlo
                        nc.tensor.matmul(
                            po[:, i * 65:(i + 1) * 65],
                            lhsT=eS[:, col0 + i * w:
                                    col0 + i * w + 128],
                            rhs=vb[:, kb * 65: kb * 65 + 65],
                            start=(kb == 0), stop=(kb == qb))
                rc = recp.tile([128, 2], f32, tag="rc",
                               name=f"rc{pair}_{qb}")
                nc.vector.reciprocal(
                    rc[:], po[:].rearrange("p (h c) -> p h c",
                                           h=2)[:, :, 64])
                an = anat.tile([128, 128], bf16, tag="an",
                               name=f"an{pair}_{qb}")
                # single fused normalize: rc column broadcast over each
                # head's 64 output columns (stride-0 free dim)
                rcb = rc[:].rearrange(
                    "p (h j) -> p h j", j=1).broadcast_to((128, 2, 64))
                nc.vector.tensor_tensor(
                    an[:].rearrange("p (h j) -> p h j", h=2),
                    po[:].rearrange("p (h c) -> p h c", h=2)[:, :, 0:64],
                    rcb, mult)
                an_t[(pair, qb)] = an

            def emit_pv_fin(pair, qb):
                pt = pvp.tile([128, 128], bf16, tag="po",
                              name=f"pt{pair}_{qb}")
                an = an_t.pop((pair, qb))
                nc.tensor.transpose(pt[:], an[:], idn[:])
                if qb >= 14:
                    nc.scalar.copy(
                        attnT[pair][:, qb * 128:(qb + 1) * 128], pt[:])
                else:
                    nc.vector.tensor_copy(
                        attnT[pair][:, qb * 128:(qb + 1) * 128], pt[:])

            def emit_stage3(sb, split):
                nonlocal s1, yst, ypp
                if yst is None:
                    s1.close()
                    yst = ctx.enter_context(
                        tc.tile_pool(name="ystage", bufs=4))
                    ypp = ctx.enter_context(
                        tc.tile_pool(name="ypp", bufs=2, space="PSUM"))
                ys = yst.tile([128, D], bf16, tag="ys", name=f"ys{sb}")
                # separate half-tiles so half c2=1's matmuls don't WAR on
                # half c2=0's PSUM->SBUF copies (deps are tile-granular)
                for c2 in range(2):
                    yph = ypp.tile([128, 512], f32, tag="yp",
                                   name=f"yp{sb}_{c2}")
                    for hp in range(2):
                        nc.tensor.matmul(
                            yph[:],
                            lhsT=attnT[hp][:, sb * 128:(sb + 1) * 128],
                            rhs=woT[:, hp * D + c2 * 512:
                                    hp * D + (c2 + 1) * 512],
                            start=(hp == 0), stop=(hp == 1))
                    lo = c2 * 512
                    if split:
                        # tail: halves on alternating copy engines and DMA
                        # dispatch queues; the very last half goes out in
                        # quarters so the final copy->DMA->sem chain is short
                        if c2 == 0:
                            nc.vector.tensor_copy(
                                ys[:, lo:lo + 512], yph[:])
                            nc.sync.dma_start(
                                y_d[sb * 128:(sb + 1) * 128, lo:lo + 512],
                                ys[:, lo:lo + 512])
                        else:
                            nc.vector.tensor_copy(
                                ys[:, lo:lo + 256], yph[:, 0:256])
                            nc.sync.dma_start(
                                y_d[sb * 128:(sb + 1) * 128, lo:lo + 256],
                                ys[:, lo:lo + 256])
                            nc.scalar.copy(
                                ys[:, lo + 256:lo + 512], yph[:, 256:512])
                            nc.scalar.dma_start(
                                y_d[sb * 128:(sb + 1) * 128,
                                    lo + 256:lo + 512],
                                ys[:, lo + 256:lo + 512])
                    else:
                        nc.vector.tensor_copy(
                            ys[:, lo:lo + 512], yph[:])
                        if c2 == 1:
                            nc.sync.dma_start(
                                y_d[sb * 128:(sb + 1) * 128, :], ys[:])

            # ---- run the schedule ----
            emitters = {
                'proj': emit_proj, 'proja': emit_proj_a, 'projb': emit_proj_b,
                'rope': emit_rope, 'trq': emit_trq, 'trk': emit_trk,
                'chunk': emit_chunk, 'pvm': emit_pv_mm, 'pvf': emit_pv_fin,
                'st3': emit_stage3,
            }
            for kind, args in order:
                emitters[kind](*args)

    nc.compile()
    return nc


def _get_module():
    if "nc" not in _CACHE:
        _CACHE["nc"] = _build_module()
    return _CACHE["nc"]


def _host_tables(freqs_cos, freqs_sin):
    # ctk[p, sb*32 + j] = cos[sb*128 + p, j]; q-rope reads the same table
    # with a stride-0 head broadcast on-device
    c3 = freqs_cos.reshape(NSB, 128, 32).transpose(1, 0, 2)  # [p, sb, j]
    s3 = freqs_sin.reshape(NSB, 128, 32).transpose(1, 0, 2)
    ctk = np.ascontiguousarray(c3).reshape(128, NSB * 32)
    stk = np.ascontiguousarray(s3).reshape(128, NSB * 32)
    return ctk, stk


def make_in_maps(x, wq, wk, wv, wo, freqs_cos, freqs_sin):
    x = np.asarray(x, np.float32)
    wq = np.asarray(wq, np.float32)
    wk = np.asarray(wk, np.float32)
    wv = np.asarray(wv, np.float32)
    wo = np.asarray(wo, np.float32)
    freqs_cos = np.asarray(freqs_cos, np.float32)
    freqs_sin = np.asarray(freqs_sin, np.float32)

    # deinterleave rope pairs within each head: [r0 i0 r1 i1 ...] ->
    # [r0..r31 | i0..i31]
    idx = np.concatenate([np.arange(0, HD, 2), np.arange(1, HD, 2)])
    wq_p = wq.reshape(H, HD, D)[:, idx, :].reshape(H * HD, D)
    wk_p = wk.reshape(HKV, HD, D)[:, idx, :].reshape(HKV * HD, D)

    ctk, stk = _host_tables(freqs_cos, freqs_sin)
    # 0/1 lower-triangle multiplicative mask (k <= q keeps, k > q zeroed)
    kk, qq = np.arange(128)[:, None], np.arange(128)[None, :]
    maskneg = np.where(kk <= qq, 1.0, 0.0).astype(np.float32)
    ident = np.eye(128)

    common = {
        "ctk": ctk.astype(BF16), "stk": stk.astype(BF16),
        "maskneg": maskneg.astype(BF16), "ident": ident.astype(BF16),
    }
    xT_b = [np.ascontiguousarray(x[b].T).astype(BF16) for b in range(B)]
    in_maps = []
    for core in range(N_CORES):
        b, g = divmod(core, HKV)
        wqT = wq_p[g * 256:(g + 1) * 256].T
        wkT = wk_p[g * 64:(g + 1) * 64].T
        wvT = wv[g * 64:(g + 1) * 64].T
        wcat = np.ascontiguousarray(
            np.concatenate([wqT, wkT, wvT], axis=1)).astype(BF16)
        woTg = np.ascontiguousarray(wo[:, g * 256:(g + 1) * 256].T).astype(BF16)
        in_maps.append({"xT": xT_b[b], "wcatT": wcat, "woT": woTg, **common})
    return in_maps


def _causal_fast_path_ok(mask):
    m = np.asarray(mask)
    if m.shape != (S, S):
        return False
    upper = m[np.triu_indices(S, 1)]
    lower = m[np.tril_indices(S, 0)]
    return bool(np.all(upper <= -1e8) and np.all(lower == 0))


def _numpy_fallback(x, wq, wk, wv, wo, freqs_cos, freqs_sin, mask):
    x = np.asarray(x, np.float32)
    xq = (x.reshape(B * S, D) @ np.asarray(wq, np.float32).T).reshape(B, S, H, HD)
    xk = (x.reshape(B * S, D) @ np.asarray(wk, np.float32).T).reshape(B, S, HKV, HD)
    xv = (x.reshape(B * S, D) @ np.asarray(wv, np.float32).T).reshape(B, S, HKV, HD)

    def rope(t, nh):
        tf = t.reshape(B, S, nh, HD // 2, 2)
        tr, ti = tf[..., 0], tf[..., 1]
        c = np.asarray(freqs_cos, np.float32)[None, :, None, :]
        s = np.asarray(freqs_sin, np.float32)[None, :, None, :]
        outr = tr * c - ti * s
        outi = tr * s + ti * c
        return np.stack([outr, outi], axis=-1).reshape(B, S, nh, HD)

    xq = rope(xq, H)
    xk = rope(xk, HKV)
    xqg = xq.reshape(B, S, HKV, REP, HD)
    scores = np.einsum("bqgrd,bkgd->bgrqk", xqg, xk) / np.sqrt(np.float32(HD))
    scores = scores + np.asarray(mask, np.float32)[None, None, None, :, :]
    scores = scores - scores.max(axis=-1, keepdims=True)
    e = np.exp(scores)
    attn = e / e.sum(axis=-1, keepdims=True)
    out = np.einsum("bgrqk,bkgd->bqgrd", attn, xv).reshape(B, S, H * HD)
    return (out.reshape(B * S, H * HD) @ np.asarray(wo, np.float32)
            .T).reshape(B, S, D).astype(np.float32)


def kernel(x, wq, wk, wv, wo, freqs_cos, freqs_sin, mask):
    if not _causal_fast_path_ok(mask):
        return _numpy_fallback(x, wq, wk, wv, wo, freqs_cos, freqs_sin, mask)
    from concourse import bass_utils
    nc = _get_module()
    in_maps = make_in_maps(x, wq, wk, wv, wo, freqs_cos, freqs_sin)
    res = bass_utils.run_bass_kernel_spmd(nc, in_maps,
                                          core_ids=list(range(N_CORES)))
    y = np.zeros((B, S, D), np.float32)
    for core in range(N_CORES):
        b = core // HKV
        y[b] += res.results[core]["y"].astype(np.float32)
    return y
